# revision 17
# baseline (speedup 1.0000x reference)
"""Causal self-attention with relative position bias on 8 Trainium2 NeuronCores.

Sharding: core = (batch b in {0,1}) x (head-group g in {0..3}, 4 heads each).
Each core computes its QKV projection slice, attention for its 4 heads, and a
partial out-projection (its 256 rows of Wout); the host sums the 4 partials
per batch (tensor-parallel reduce done host-side).

On-chip layout is fully transposed ([channels, tokens]) so no transposes are
needed anywhere:
  - qT/kT come out of the projection as [head_dim, T] (weights stationary).
  - scores are computed as S^T = kT_tile^T-contraction -> [s, t] tiles; the
    softmax normalization runs along the partition (s) axis.
  - softmax denominators come free: V is augmented with a ones column, so row
    64 of the AV PSUM accumulator holds the per-column sums.
  - rel-bias + causal mask are folded into one multiplicative exp(bias) strip
    [128, 1024] per head (Toeplitz: every near-diagonal tile is a column
    slice); far tiles (saturated bias) use a per-head additive constant
    inside the ACT Exp.
"""
import numpy as np

import sys
if "/opt/trn_rl_repo" not in sys.path:
    sys.path.insert(0, "/opt/trn_rl_repo")

import concourse.bass as bass
from concourse import mybir
from concourse.tile import TileContext
from concourse.bass_utils import run_bass_kernel_spmd

F32 = mybir.dt.float32
F32R = mybir.dt.float32r
AFT = mybir.ActivationFunctionType

T = 2048
C = 1024
NH = 16
DH = 64
MAX_DIST = 128
HPC = 4            # heads per core
QKC = 256          # q (or k, or v) channels per core
TT = 512           # t tile
ST = 128           # s tile
N_TT = T // TT     # 4
N_ST = T // ST     # 16
N_CT = C // 128    # 8 contraction tiles

_CACHE = {}


def r(ap):
    return ap.bitcast(F32R)


# Engine-datapath instruction descriptors have exactly ONE sync-wait slot
# (NEURON_ISA_TPB_EVENTS); walrus codegen dies with "Too many sync wait
# commands" on multi-wait instructions. Peel extra waits onto preceding
# same-engine NoOps (engines execute queue entries in order, so semantics
# are identical).
_SPLIT_TYPES = {
    "InstActivation", "InstMatmult", "InstTensorTensor", "InstTensorCopy",
    "InstReciprocal", "InstMemset", "InstTensorScalarPtr", "InstTensorScalar",
    "InstTensorReduce", "InstIota", "InstCopy", "InstDMACopy", "InstDrain",
}


def _split_multi_waits(nc):
    n_split = 0
    for func in nc.m.functions:
        for blk in func.blocks:
            out = []
            for inst in blk.instructions:
                si = inst.sync_info
                if (si is not None and si.on_wait and len(si.on_wait) > 1
                        and type(inst).__name__ in _SPLIT_TYPES):
                    waits = list(si.on_wait)
                    for w in waits[:-1]:
                        out.append(mybir.InstNoOp(
                            name=f"WSPLIT-{n_split}-{len(out)}",
                            engine=inst.engine,
                            sync_info=mybir.SyncInfo(on_wait=[w], on_update=[]),
                        ))
                    inst.sync_info = mybir.SyncInfo(
                        on_wait=[waits[-1]], on_update=list(si.on_update))
                    n_split += 1
                out.append(inst)
            blk.instructions = out
    return n_split


def build_program(split_waits=True):
    nc = bass.Bass(trn_type="TRN2")

    xT = nc.dram_tensor("xT", [C, T], F32R, kind="ExternalInput")
    wqk = nc.dram_tensor("wqk", [C, 512], F32R, kind="ExternalInput")
    wv = nc.dram_tensor("wv", [C, QKC], F32R, kind="ExternalInput")
    wout = nc.dram_tensor("wout", [QKC, C], F32R, kind="ExternalInput")
    bqk = nc.dram_tensor("bqk", [128, 4], F32, kind="ExternalInput")
    bv = nc.dram_tensor("bv", [1, QKC], F32R, kind="ExternalInput")
    mask = nc.dram_tensor("mask", [HPC, 128, 1024], F32, kind="ExternalInput")
    bfar = nc.dram_tensor("bfar", [128, HPC], F32, kind="ExternalInput")
    onesc = nc.dram_tensor("onesc", [128, 128], F32R, kind="ExternalInput")
    outp = nc.dram_tensor("outp", [C, T], F32, kind="ExternalOutput")

    with TileContext(nc) as tc:
        # ---------- long-lived pools ----------
        with (
            tc.tile_pool(name="qk_res", bufs=1) as qk_res,
            tc.tile_pool(name="v_res", bufs=1) as v_res,
            tc.tile_pool(name="consts", bufs=1) as consts,
        ):
            # packed q/k pair tiles: [128, T]; pair hp holds heads 2hp (rows
            # 0-63) and 2hp+1 (rows 64-127)
            q_sb = [qk_res.tile([128, T], F32R, tag=f"q{hp}", name=f"q{hp}")
                    for hp in range(2)]
            k_sb = [qk_res.tile([128, T], F32R, tag=f"k{hp}", name=f"k{hp}")
                    for hp in range(2)]
            # v augmented: [128 s, 16 stiles, 4 heads, 65]
            v_sb = v_res.tile([128, N_ST, HPC, 65], F32R)

            mask_sb = consts.tile([128, HPC, 1024], F32)
            nc.sync.dma_start(out=mask_sb, in_=mask.rearrange("h p u -> p h u"))
            bfar_sb = consts.tile([128, HPC], F32)
            nc.sync.dma_start(out=bfar_sb, in_=bfar[:, :])
            bqk_sb = consts.tile([128, 4], F32)
            nc.sync.dma_start(out=bqk_sb, in_=bqk[:, :])
            bv_sb = consts.tile([1, QKC], F32R)
            nc.sync.dma_start(out=bv_sb, in_=bv[:, :])
            wout_sb = consts.tile([128, 2, 8, 128], F32R)
            nc.sync.dma_start(
                out=wout_sb,
                in_=wout.rearrange("(hp p) (o m) -> p hp o m", p=128, m=128),
            )
            ones_sb = consts.tile([1, 128], F32R)
            nc.sync.dma_start(out=ones_sb, in_=onesc[0:1, :])
            # ones column of v_aug (memset cannot write f32r: ISA check)
            nc.sync.dma_start(
                out=v_sb[:, :, :, 64:65],
                in_=onesc[:, 0:64].rearrange("p (i h o) -> p i h o",
                                             i=N_ST, h=HPC, o=1),
            )

            # ---------- phase A: projections (xT/W resident only here) ------
            with (
                tc.tile_pool(name="xw", bufs=1) as xw,
                tc.tile_pool(name="pproj", bufs=2, space="PSUM") as pproj,
            ):
                xT_sb = [xw.tile([128, T], F32R, tag=f"x{i}", name=f"x{i}")
                         for i in range(N_CT)]
                for i in range(N_CT):
                    nc.sync.dma_start(out=xT_sb[i], in_=xT[128 * i:128 * (i + 1), :])
                wqk_sb = [xw.tile([128, 512], F32R, tag=f"w{i}", name=f"w{i}")
                          for i in range(N_CT)]
                for i in range(N_CT):
                    nc.sync.dma_start(out=wqk_sb[i], in_=wqk[128 * i:128 * (i + 1), :])
                wv_sb = xw.tile([128, N_CT, QKC], F32R)
                nc.sync.dma_start(
                    out=wv_sb, in_=wv.rearrange("(i p) n -> p i n", p=128)
                )

                # q/k projection: out [128 qc-block, 512 t], W stationary
                for blk in range(4):                      # q01 q23 k01 k23
                    dst = (q_sb if blk < 2 else k_sb)[blk % 2]
                    for tt in range(N_TT):
                        ps = pproj.tile([128, TT], F32, tag="pqk")
                        for ci in range(N_CT):
                            nc.tensor.matmul(
                                ps,
                                lhsT=(wqk_sb[ci][:, 128 * blk:128 * (blk + 1)]),
                                rhs=(xT_sb[ci][:, TT * tt:TT * (tt + 1)]),
                                start=(ci == 0),
                                stop=(ci == N_CT - 1),
                            )
                        # PSUM -> SBUF copy with per-channel bias add
                        nc.scalar.activation(
                            out=dst[:, TT * tt:TT * (tt + 1)],
                            in_=ps,
                            func=AFT.Identity,
                            bias=bqk_sb[:, blk:blk + 1],
                        )

                # v projection: out [128 s, 256 vc], xT stationary
                for st in range(N_ST):
                    ps = pproj.tile([128, QKC], F32, tag="pv")
                    for ci in range(N_CT):
                        nc.tensor.matmul(
                            ps,
                            lhsT=(xT_sb[ci][:, ST * st:ST * (st + 1)]),
                            rhs=(wv_sb[:, ci, :]),
                            start=(ci == 0),
                            stop=False,
                        )
                    # + ones_s x bv outer product adds the v bias
                    nc.tensor.matmul(
                        ps,
                        lhsT=(ones_sb[:, 0:128]),
                        rhs=(bv_sb),
                        start=False,
                        stop=True,
                    )
                    # interleave into v_aug layout [st][h][0:64]
                    nc.vector.tensor_copy(
                        out=v_sb[:, st, :, 0:64],
                        in_=ps.rearrange("p (h d) -> p h d", h=HPC),
                    )

            # ---------- phase B: attention + out projection ----------
            with (
                tc.tile_pool(name="epool", bufs=4) as epool,
                tc.tile_pool(name="onorm", bufs=2) as onorm,
                tc.tile_pool(name="ostage", bufs=3) as ostage,
                tc.tile_pool(name="rpool", bufs=2) as rpool,
                tc.tile_pool(name="pqk", bufs=3, space="PSUM") as pqk,
                tc.tile_pool(name="pav", bufs=2, space="PSUM") as pav,
                tc.tile_pool(name="pbc", bufs=1, space="PSUM") as pbc,
                tc.tile_pool(name="pout", bufs=2, space="PSUM") as pout,
            ):
                for tt in range(N_TT):
                    t0 = TT * tt
                    on_pair = [onorm.tile([128, TT], F32R, tag=f"on{hp}",
                                          name=f"on{hp}")
                               for hp in range(2)]
                    for hp in range(2):
                        for s in range(2):
                            h = 2 * hp + s
                            pb = 64 * s  # partition base within pair tiles
                            ps_av = pav.tile([65, TT], F32, tag="av")
                            nj = 4 * tt + 4
                            for j in range(nj):
                                s0 = ST * j
                                d0 = s0 - t0
                                ps_qk = pqk.tile([128, TT], F32, tag="qk")
                                nc.tensor.matmul(
                                    ps_qk,
                                    lhsT=(k_sb[hp][pb:pb + 64, s0:s0 + ST]),
                                    rhs=(q_sb[hp][pb:pb + 64, t0:t0 + TT]),
                                    start=True,
                                    stop=True,
                                )
                                e = epool.tile([128, TT], F32R, tag="e")
                                if d0 <= -256:
                                    nc.scalar.activation(
                                        out=e, in_=ps_qk, func=AFT.Exp,
                                        bias=bfar_sb[:, h:h + 1],
                                    )
                                else:
                                    nc.scalar.activation(
                                        out=e, in_=ps_qk, func=AFT.Exp)
                                    nc.vector.tensor_mul(
                                        e, e,
                                        mask_sb[:, h, 384 - d0:384 - d0 + TT],
                                    )
                                nc.tensor.matmul(
                                    ps_av,
                                    lhsT=(v_sb[:, j, h, :]),
                                    rhs=(e),
                                    start=(j == 0),
                                    stop=(j == nj - 1),
                                )
                            # normalize: rows 0-63 / row 64
                            recip = rpool.tile([1, TT], F32R, tag="rc")
                            with nc.allow_low_precision(
                                    reason="f32r out for PE broadcast"):
                                nc.vector.reciprocal(recip, ps_av[64:65, :])
                            ps_bc = pbc.tile([64, TT], F32, tag="bc")
                            nc.tensor.matmul(
                                ps_bc,
                                lhsT=(ones_sb[:, 0:64]),
                                rhs=(recip),
                                start=True,
                                stop=True,
                            )
                            # DVE can read only one PSUM operand: stage the
                            # AV rows through SBUF first (ACT sits near PSUM)
                            otmp = rpool.tile([64, TT], F32, tag="ot")
                            nc.scalar.copy(otmp, ps_av[0:64, :])
                            nc.vector.tensor_mul(
                                on_pair[hp][pb:pb + 64, :],
                                otmp,
                                ps_bc,
                            )
                    # out projection for this t tile
                    for oc in range(8):
                        ps_o = pout.tile([128, TT], F32, tag="po")
                        for hp in range(2):
                            nc.tensor.matmul(
                                ps_o,
                                lhsT=(wout_sb[:, hp, oc, :]),
                                rhs=(on_pair[hp]),
                                start=(hp == 0),
                                stop=(hp == 1),
                            )
                        stg = ostage.tile([128, TT], F32, tag="st")
                        nc.scalar.copy(stg, ps_o)
                        nc.sync.dma_start(
                            out=outp[128 * oc:128 * (oc + 1), t0:t0 + TT],
                            in_=stg,
                        )
    if split_waits:
        _split_multi_waits(nc)
    return nc


def build_strips(rel_emb):
    """[NH, 128, 1024] f32: strip[h][i, u] = exp(bias(d)), d = i - u + 384,
    zero where d > 0 (causal)."""
    i = np.arange(128)[:, None]
    u = np.arange(1024)[None, :]
    d = i - u + 384
    idx = np.clip(d, -MAX_DIST, MAX_DIST) + MAX_DIST        # [128, 1024]
    strips = np.exp(rel_emb[idx, :]).astype(np.float32)     # [128, 1024, NH]
    strips[d > 0, :] = 0.0
    return np.ascontiguousarray(strips.transpose(2, 0, 1))


def prepare_in_maps(x, Wqkv, bqkv, Wout, bout, rel_emb):
    x = np.asarray(x, np.float32)
    Wqkv = np.asarray(Wqkv, np.float32)
    bqkv = np.asarray(bqkv, np.float32)
    Wout = np.asarray(Wout, np.float32)
    bout = np.asarray(bout, np.float32)
    rel_emb = np.asarray(rel_emb, np.float32)

    scale = np.float32(np.sqrt(DH))
    strips = build_strips(rel_emb)

    in_maps = []
    for core in range(8):
        b, g = divmod(core, 4)
        sl = slice(QKC * g, QKC * (g + 1))
        wq = (Wqkv[:, sl] / scale).astype(np.float32)
        wk = Wqkv[:, 1024 + QKC * g:1024 + QKC * (g + 1)]
        bq = (bqkv[sl] / scale).astype(np.float32)
        bk = bqkv[1024 + QKC * g:1024 + QKC * (g + 1)]
        bqk_arr = np.stack(
            [bq[:128], bq[128:], bk[:128], bk[128:]], axis=1
        ).astype(np.float32)
        in_maps.append({
            "xT": np.ascontiguousarray(x[b].T),
            "wqk": np.ascontiguousarray(
                np.concatenate([wq, wk], axis=1)),
            "wv": np.ascontiguousarray(
                Wqkv[:, 2048 + QKC * g:2048 + QKC * (g + 1)]),
            "wout": np.ascontiguousarray(Wout[sl, :]),
            "bqk": np.ascontiguousarray(bqk_arr),
            "bv": np.ascontiguousarray(
                bqkv[2048 + QKC * g:2048 + QKC * (g + 1)][None, :]),
            "mask": np.ascontiguousarray(strips[HPC * g:HPC * (g + 1)]),
            "bfar": np.ascontiguousarray(
                np.broadcast_to(rel_emb[0, HPC * g:HPC * (g + 1)][None, :],
                                (128, HPC))),
            "onesc": np.ones((128, 128), np.float32),
        })
    return in_maps


def _gather(res, bout):
    out = np.empty((2, T, C), np.float32)
    for b in range(2):
        acc = res.results[4 * b]["outp"].astype(np.float32)
        for g in range(1, 4):
            acc = acc + res.results[4 * b + g]["outp"]
        out[b] = acc.T + np.asarray(bout, np.float32)[None, :]
    return out


def _get_nc():
    if "nc" not in _CACHE:
        _CACHE["nc"] = build_program()
    return _CACHE["nc"]


def kernel(x, Wqkv, bqkv, Wout, bout, rel_emb):
    in_maps = prepare_in_maps(x, Wqkv, bqkv, Wout, bout, rel_emb)
    res = run_bass_kernel_spmd(_get_nc(), in_maps, core_ids=list(range(8)))
    return _gather(res, bout)


def run_traced(inputs, tmpdir=None):
    """Run with NTFF profiling; returns BassKernelResults (exec_time_ns etc.)."""
    in_maps = prepare_in_maps(**inputs)
    res = run_bass_kernel_spmd(
        _get_nc(), in_maps, core_ids=list(range(8)), trace=True, tmpdir=tmpdir
    )
    return res


# revision 19
# speedup vs baseline: 1.4539x; 1.4539x over previous
"""Causal self-attention with relative position bias on 8 Trainium2 NeuronCores.

Sharding: core = (batch b in {0,1}) x (head-group g in {0..3}, 4 heads each).
Each core computes its QKV projection slice, attention for its 4 heads, and a
partial out-projection (its 256 rows of Wout); the host sums the 4 partials
per batch (tensor-parallel reduce done host-side).

On-chip layout is fully transposed ([channels, tokens]) so no transposes are
needed anywhere:
  - qT/kT come out of the projection as [head_dim, T] (weights stationary).
  - scores are computed as S^T -> [s, t] tiles; the softmax normalization
    runs along the partition (s) axis.
  - softmax denominators come free: V is augmented with a ones column, so row
    64 of the AV PSUM accumulator holds the per-column sums.
  - rel-bias + causal mask are folded into one multiplicative exp(bias) strip
    [128, 1024] per head (Toeplitz: every near-diagonal tile is a column
    slice); far tiles (saturated bias) use a per-head additive constant
    inside the ACT Exp.

Matmul operands are bf16 (full PE rate; fp32 moving operands stream at ~2
cycles/row). PSUM accumulation is fp32, and the softmax normalization chain
(sums -> 1/sum -> broadcast) stays fp32/f32r.
"""
import numpy as np
import ml_dtypes

import sys
if "/opt/trn_rl_repo" not in sys.path:
    sys.path.insert(0, "/opt/trn_rl_repo")

import concourse.bass as bass
from concourse import mybir
from concourse.tile import TileContext
from concourse.bass_utils import run_bass_kernel_spmd

F32 = mybir.dt.float32
F32R = mybir.dt.float32r
BF = mybir.dt.float16   # fp16: same PE rate as bf16, 8x finer mantissa
AFT = mybir.ActivationFunctionType
BF_NP = np.float16

T = 2048
C = 1024
NH = 16
DH = 64
MAX_DIST = 128
HPC = 4            # heads per core
QKC = 256          # q (or k, or v) channels per core
TT = 512           # t tile
ST = 128           # s tile
N_TT = T // TT     # 4
N_ST = T // ST     # 16
N_CT = C // 128    # 8 contraction tiles

_CACHE = {}


# Engine-datapath instruction descriptors have exactly ONE sync-wait slot
# (NEURON_ISA_TPB_EVENTS); walrus codegen dies with "Too many sync wait
# commands" on multi-wait instructions. Peel extra waits onto preceding
# same-engine NoOps (engines execute queue entries in order, so semantics
# are identical).
_SPLIT_TYPES = {
    "InstActivation", "InstMatmult", "InstTensorTensor", "InstTensorCopy",
    "InstReciprocal", "InstMemset", "InstTensorScalarPtr", "InstTensorScalar",
    "InstTensorReduce", "InstIota", "InstCopy", "InstDMACopy", "InstDrain",
}


def _split_multi_waits(nc):
    n_split = 0
    for func in nc.m.functions:
        for blk in func.blocks:
            out = []
            for inst in blk.instructions:
                si = inst.sync_info
                if (si is not None and si.on_wait and len(si.on_wait) > 1
                        and type(inst).__name__ in _SPLIT_TYPES):
                    waits = list(si.on_wait)
                    for w in waits[:-1]:
                        out.append(mybir.InstNoOp(
                            name=f"WSPLIT-{n_split}-{len(out)}",
                            engine=inst.engine,
                            sync_info=mybir.SyncInfo(on_wait=[w], on_update=[]),
                        ))
                    inst.sync_info = mybir.SyncInfo(
                        on_wait=[waits[-1]], on_update=list(si.on_update))
                    n_split += 1
                out.append(inst)
            blk.instructions = out
    return n_split


def build_program(split_waits=True):
    nc = bass.Bass(trn_type="TRN2")

    xT = nc.dram_tensor("xT", [C, T], BF, kind="ExternalInput")
    wqk = nc.dram_tensor("wqk", [C, 512], BF, kind="ExternalInput")
    wv = nc.dram_tensor("wv", [C, QKC], BF, kind="ExternalInput")
    wout = nc.dram_tensor("wout", [QKC, C], BF, kind="ExternalInput")
    bqk = nc.dram_tensor("bqk", [128, 4], F32, kind="ExternalInput")
    bv = nc.dram_tensor("bv", [1, QKC], BF, kind="ExternalInput")
    mask = nc.dram_tensor("mask", [HPC, 128, 1024], BF, kind="ExternalInput")
    bfar = nc.dram_tensor("bfar", [128, HPC], F32, kind="ExternalInput")
    onesc = nc.dram_tensor("onesc", [128, 128], BF, kind="ExternalInput")
    onesr = nc.dram_tensor("onesr", [1, 64], F32R, kind="ExternalInput")
    outp = nc.dram_tensor("outp", [C, T], F32, kind="ExternalOutput")

    with TileContext(nc) as tc:
        # ---------- long-lived pools ----------
        with (
            tc.tile_pool(name="qk_res", bufs=1) as qk_res,
            tc.tile_pool(name="v_res", bufs=1) as v_res,
            tc.tile_pool(name="consts", bufs=1) as consts,
        ):
            # packed q/k pair tiles: [128, T]; pair hp holds heads 2hp (rows
            # 0-63) and 2hp+1 (rows 64-127)
            q_sb = [qk_res.tile([128, T], BF, tag=f"q{hp}", name=f"q{hp}")
                    for hp in range(2)]
            k_sb = [qk_res.tile([128, T], BF, tag=f"k{hp}", name=f"k{hp}")
                    for hp in range(2)]
            # v augmented: [128 s, 16 stiles, 4 heads, 65]
            v_sb = v_res.tile([128, N_ST, HPC, 65], BF)

            mask_sb = consts.tile([128, HPC, 1024], BF)
            nc.sync.dma_start(out=mask_sb, in_=mask.rearrange("h p u -> p h u"))
            bfar_sb = consts.tile([128, HPC], F32)
            nc.sync.dma_start(out=bfar_sb, in_=bfar[:, :])
            bqk_sb = consts.tile([128, 4], F32)
            nc.sync.dma_start(out=bqk_sb, in_=bqk[:, :])
            bv_sb = consts.tile([1, QKC], BF)
            nc.sync.dma_start(out=bv_sb, in_=bv[:, :])
            wout_sb = consts.tile([128, 2, 8, 128], BF)
            nc.sync.dma_start(
                out=wout_sb,
                in_=wout.rearrange("(hp p) (o m) -> p hp o m", p=128, m=128),
            )
            ones_sb = consts.tile([1, 128], BF)
            nc.sync.dma_start(out=ones_sb, in_=onesc[0:1, :])
            onesr_sb = consts.tile([1, 64], F32R)
            nc.sync.dma_start(out=onesr_sb, in_=onesr[:, :])
            # ones column of v_aug
            nc.sync.dma_start(
                out=v_sb[:, :, :, 64:65],
                in_=onesc[:, 0:64].rearrange("p (i h o) -> p i h o",
                                             i=N_ST, h=HPC, o=1),
            )

            # ---------- phase A: projections (xT/W resident only here) ------
            with (
                tc.tile_pool(name="xw", bufs=1) as xw,
                tc.tile_pool(name="pproj", bufs=2, space="PSUM") as pproj,
            ):
                xT_sb = [xw.tile([128, T], BF, tag=f"x{i}", name=f"x{i}")
                         for i in range(N_CT)]
                for i in range(N_CT):
                    nc.sync.dma_start(out=xT_sb[i], in_=xT[128 * i:128 * (i + 1), :])
                wqk_sb = [xw.tile([128, 512], BF, tag=f"w{i}", name=f"w{i}")
                          for i in range(N_CT)]
                for i in range(N_CT):
                    nc.sync.dma_start(out=wqk_sb[i], in_=wqk[128 * i:128 * (i + 1), :])
                wv_sb = xw.tile([128, N_CT, QKC], BF)
                nc.sync.dma_start(
                    out=wv_sb, in_=wv.rearrange("(i p) n -> p i n", p=128)
                )

                # q/k projection: out [128 qc-block, 512 t], W stationary
                for blk in range(4):                      # q01 q23 k01 k23
                    dst = (q_sb if blk < 2 else k_sb)[blk % 2]
                    for tt in range(N_TT):
                        ps = pproj.tile([128, TT], F32, tag="pqk")
                        for ci in range(N_CT):
                            nc.tensor.matmul(
                                ps,
                                lhsT=wqk_sb[ci][:, 128 * blk:128 * (blk + 1)],
                                rhs=xT_sb[ci][:, TT * tt:TT * (tt + 1)],
                                start=(ci == 0),
                                stop=(ci == N_CT - 1),
                            )
                        # PSUM -> SBUF copy with per-channel bias add
                        nc.scalar.activation(
                            out=dst[:, TT * tt:TT * (tt + 1)],
                            in_=ps,
                            func=AFT.Identity,
                            bias=bqk_sb[:, blk:blk + 1],
                        )

                # v projection: out [128 s, 256 vc], xT stationary
                for st in range(N_ST):
                    ps = pproj.tile([128, QKC], F32, tag="pv")
                    for ci in range(N_CT):
                        nc.tensor.matmul(
                            ps,
                            lhsT=xT_sb[ci][:, ST * st:ST * (st + 1)],
                            rhs=wv_sb[:, ci, :],
                            start=(ci == 0),
                            stop=False,
                        )
                    # + ones_s x bv outer product adds the v bias
                    nc.tensor.matmul(
                        ps,
                        lhsT=ones_sb[:, 0:128],
                        rhs=bv_sb,
                        start=False,
                        stop=True,
                    )
                    # interleave into v_aug layout [st][h][0:64]
                    nc.vector.tensor_copy(
                        out=v_sb[:, st, :, 0:64],
                        in_=ps.rearrange("p (h d) -> p h d", h=HPC),
                    )

            # ---------- phase B: attention + out projection ----------
            with (
                tc.tile_pool(name="epool", bufs=4) as epool,
                tc.tile_pool(name="onorm", bufs=2) as onorm,
                tc.tile_pool(name="ostage", bufs=3) as ostage,
                tc.tile_pool(name="rpool", bufs=2) as rpool,
                tc.tile_pool(name="pqk", bufs=3, space="PSUM") as pqk,
                tc.tile_pool(name="pav", bufs=2, space="PSUM") as pav,
                tc.tile_pool(name="pbc", bufs=1, space="PSUM") as pbc,
                tc.tile_pool(name="pout", bufs=2, space="PSUM") as pout,
            ):
                for tt in range(N_TT):
                    t0 = TT * tt
                    on_pair = [onorm.tile([128, TT], BF, tag=f"on{hp}",
                                          name=f"on{hp}")
                               for hp in range(2)]
                    for hp in range(2):
                        for s in range(2):
                            h = 2 * hp + s
                            pb = 64 * s  # partition base within pair tiles
                            ps_av = pav.tile([65, TT], F32, tag="av")
                            nj = 4 * tt + 4
                            for j in range(nj):
                                s0 = ST * j
                                d0 = s0 - t0
                                ps_qk = pqk.tile([128, TT], F32, tag="qk")
                                nc.tensor.matmul(
                                    ps_qk,
                                    lhsT=k_sb[hp][pb:pb + 64, s0:s0 + ST],
                                    rhs=q_sb[hp][pb:pb + 64, t0:t0 + TT],
                                    start=True,
                                    stop=True,
                                )
                                e = epool.tile([128, TT], BF, tag="e")
                                if d0 <= -256:
                                    nc.scalar.activation(
                                        out=e, in_=ps_qk, func=AFT.Exp,
                                        bias=bfar_sb[:, h:h + 1],
                                    )
                                else:
                                    nc.scalar.activation(
                                        out=e, in_=ps_qk, func=AFT.Exp)
                                    nc.vector.tensor_mul(
                                        e, e,
                                        mask_sb[:, h, 384 - d0:384 - d0 + TT],
                                    )
                                nc.tensor.matmul(
                                    ps_av,
                                    lhsT=v_sb[:, j, h, :],
                                    rhs=e,
                                    start=(j == 0),
                                    stop=(j == nj - 1),
                                )
                            # normalize rows 0-63 by row 64:
                            # 1/sum via ACT exp(-ln(x)) (DVE reciprocal on a
                            # [1,512] strip costs ~3.4us - one lane); then
                            # broadcast along partitions with a K=1 matmul.
                            lns = rpool.tile([1, TT], F32, tag="ln")
                            nc.scalar.activation(lns, ps_av[64:65, :], AFT.Ln)
                            recip = rpool.tile([1, TT], F32R, tag="rc")
                            nc.scalar.activation(recip, lns, AFT.Exp,
                                                 scale=-1.0)
                            ps_bc = pbc.tile([64, TT], F32, tag="bc")
                            nc.tensor.matmul(
                                ps_bc,
                                lhsT=onesr_sb,
                                rhs=recip,
                                start=True,
                                stop=True,
                            )
                            # DVE can read only one PSUM operand: stage the
                            # AV rows through SBUF first
                            otmp = rpool.tile([64, TT], F32, tag="ot")
                            nc.vector.tensor_copy(otmp, ps_av[0:64, :])
                            nc.vector.tensor_mul(
                                on_pair[hp][pb:pb + 64, :],
                                otmp,
                                ps_bc,
                            )
                    # out projection for this t tile
                    for oc in range(8):
                        ps_o = pout.tile([128, TT], F32, tag="po")
                        for hp in range(2):
                            nc.tensor.matmul(
                                ps_o,
                                lhsT=wout_sb[:, hp, oc, :],
                                rhs=on_pair[hp],
                                start=(hp == 0),
                                stop=(hp == 1),
                            )
                        stg = ostage.tile([128, TT], F32, tag="st")
                        nc.vector.tensor_copy(stg, ps_o)
                        nc.sync.dma_start(
                            out=outp[128 * oc:128 * (oc + 1), t0:t0 + TT],
                            in_=stg,
                        )
    if split_waits:
        _split_multi_waits(nc)
    return nc


def build_strips(rel_emb):
    """[NH, 128, 1024] f32: strip[h][i, u] = exp(bias(d)), d = i - u + 384,
    zero where d > 0 (causal)."""
    i = np.arange(128)[:, None]
    u = np.arange(1024)[None, :]
    d = i - u + 384
    idx = np.clip(d, -MAX_DIST, MAX_DIST) + MAX_DIST        # [128, 1024]
    strips = np.exp(rel_emb[idx, :]).astype(np.float32)     # [128, 1024, NH]
    strips[d > 0, :] = 0.0
    return np.ascontiguousarray(strips.transpose(2, 0, 1))


def prepare_in_maps(x, Wqkv, bqkv, Wout, bout, rel_emb):
    x = np.asarray(x, np.float32)
    Wqkv = np.asarray(Wqkv, np.float32)
    bqkv = np.asarray(bqkv, np.float32)
    Wout = np.asarray(Wout, np.float32)
    bout = np.asarray(bout, np.float32)
    rel_emb = np.asarray(rel_emb, np.float32)

    scale = np.float32(np.sqrt(DH))
    strips = build_strips(rel_emb)

    in_maps = []
    for core in range(8):
        b, g = divmod(core, 4)
        sl = slice(QKC * g, QKC * (g + 1))
        wq = (Wqkv[:, sl] / scale).astype(np.float32)
        wk = Wqkv[:, 1024 + QKC * g:1024 + QKC * (g + 1)]
        bq = (bqkv[sl] / scale).astype(np.float32)
        bk = bqkv[1024 + QKC * g:1024 + QKC * (g + 1)]
        bqk_arr = np.stack(
            [bq[:128], bq[128:], bk[:128], bk[128:]], axis=1
        ).astype(np.float32)
        in_maps.append({
            "xT": np.ascontiguousarray(x[b].T).astype(BF_NP),
            "wqk": np.ascontiguousarray(
                np.concatenate([wq, wk], axis=1)).astype(BF_NP),
            "wv": np.ascontiguousarray(
                Wqkv[:, 2048 + QKC * g:2048 + QKC * (g + 1)]).astype(BF_NP),
            "wout": np.ascontiguousarray(Wout[sl, :]).astype(BF_NP),
            "bqk": np.ascontiguousarray(bqk_arr),
            "bv": np.ascontiguousarray(
                bqkv[2048 + QKC * g:2048 + QKC * (g + 1)][None, :]).astype(BF_NP),
            "mask": np.ascontiguousarray(
                strips[HPC * g:HPC * (g + 1)]).astype(BF_NP),
            "bfar": np.ascontiguousarray(
                np.broadcast_to(rel_emb[0, HPC * g:HPC * (g + 1)][None, :],
                                (128, HPC))),
            "onesc": np.ones((128, 128), BF_NP),
            "onesr": np.ones((1, 64), np.float32),
        })
    return in_maps


def _gather(res, bout):
    out = np.empty((2, T, C), np.float32)
    for b in range(2):
        acc = res.results[4 * b]["outp"].astype(np.float32)
        for g in range(1, 4):
            acc = acc + res.results[4 * b + g]["outp"]
        out[b] = acc.T + np.asarray(bout, np.float32)[None, :]
    return out


def _get_nc():
    if "nc" not in _CACHE:
        _CACHE["nc"] = build_program()
    return _CACHE["nc"]


def kernel(x, Wqkv, bqkv, Wout, bout, rel_emb):
    in_maps = prepare_in_maps(x, Wqkv, bqkv, Wout, bout, rel_emb)
    res = run_bass_kernel_spmd(_get_nc(), in_maps, core_ids=list(range(8)))
    return _gather(res, bout)


def run_traced(inputs, tmpdir=None):
    """Run with NTFF profiling; returns BassKernelResults (exec_time_ns etc.)."""
    in_maps = prepare_in_maps(**inputs)
    res = run_bass_kernel_spmd(
        _get_nc(), in_maps, core_ids=list(range(8)), trace=True, tmpdir=tmpdir
    )
    return res
